# revision 4
# baseline (speedup 1.0000x reference)
"""Trainium2 Bass kernel for truncated BCH on 3D vector fields (v3).

Math (matches the jax reference):
  out_i = l_i + r_i + 0.25 * sum_j ( D_j l_i * r_j  -  D_j r_i * l_j )
with D_j v = v[.+1] - v[.-1] (circulant wrap) along spatial axis j.

Host re-parameterization (the key algebraic fold):
  U = (l + r)/2,  V = (r - l)/4      (both fp16)
  out_i = 2*U_i + sum_j ( D_j U_i * V_j  -  D_j V_i * U_j )
The 0.25 is absorbed into V (each bracket term is linear in V), the final
scale disappears, and the l+r linear term becomes 2*U (injected by the
final combine), so no separate s=l+r op is needed.

Sharding: 8 cores = 2 batches x 4 X-slabs of 32 planes (+1 halo plane each
side, wrapped).  Host lays data per core as one uv tensor
[Y, 2(side: 0=V 1=U), D, xh=34, ZP=132] fp16 so SBUF partition dim = Y and
all DMA runs are long/contiguous.  Output is fp16 [Y, D, 32, Z] (upcast on
host; rel-err budget is 2e-2, fp16 out costs ~1.5e-4).

Per-core engine split:
  - TensorE : Y-diffs as circulant shift-difference matmuls (dyT for the
              U side, -dyT for the V side) + accumulation of the 6 product
              terms into a PSUM accumulator via identity matmuls (+ a 2I
              inject of U on chunks that use the Act-evac combine).
  - VectorE : X-diffs (shifted-AP subtract, channel-fused), the fused
              X-products, the Y-products (from Act-evacuated fp16 dy), and
              part of the Z-side work.
  - GPSIMD  : Z-diffs, part of the Z-products, and the final combine
              out = 2*U + acc as a fused scalar_tensor_tensor op reading
              PSUM directly.
  - ScalarE : dy PSUM->SBUF fp16 evacuation (+ final evac on inject
              chunks).
All signs are folded so every accumulating matmul uses +I: V-side diffs
are computed reversed (b-a) and the V-side Y-diff uses -dyT.
"""

import sys

sys.path.insert(0, "/opt/trn_rl_repo")

import numpy as np

import concourse.bass as bass
import concourse.bacc as bacc
import concourse.mybir as mybir
import concourse.tile as tile
from concourse.bass_utils import run_bass_kernel_spmd

B, D, X, Y, Z = 2, 3, 128, 128, 128
NCORES = 8
XS = (B * X) // NCORES  # 32 output x-planes per core
ZP = Z + 4              # z padded: [z126, z127, z0..z127, z0, z1]
KB = 8                  # x-planes per item (big fused elementwise ops)
KX = 4                  # x-planes per psum chunk (acc bank = 512 f32)

F16 = mybir.dt.float16
F32 = mybir.dt.float32


def _make_wmats() -> np.ndarray:
    """[dyT | -dyT | I | 2I] as one (Y, 4Y) fp16 matrix (lhsT layout).

    matmul(out, lhsT, rhs) computes lhsT.T @ rhs.  We want Dy @ v with
    Dy[y, y'] = delta(y'=y+1) - delta(y'=y-1) (wrap), so lhsT = Dy.T.
    """
    e = np.eye(Y, dtype=np.float32)
    dy = np.roll(e, -1, axis=0) - np.roll(e, 1, axis=0)
    dyt = dy.T
    mats = np.concatenate([dyt, -dyt, e, 2.0 * e], axis=1)
    return mats.astype(np.float16)


def build_nc(xs: int = XS) -> bass.Bass:
    xh = xs + 2
    n_items = xs // KB
    n_h = KB // KX
    nc = bacc.Bacc(None)

    uv_h = nc.declare_dram_parameter("uv", [Y, 2, D, xh, ZP], F16, isOutput=False)
    w_h = nc.declare_dram_parameter("wmats", [Y, 4 * Y], F16, isOutput=False)
    out_h = nc.declare_dram_parameter("out", [Y, D, xs, Z], F16, isOutput=True)

    with tile.TileContext(nc) as tc:
        with (
            tc.tile_pool(name="inp", bufs=1) as inp,
            tc.tile_pool(name="wp", bufs=1) as wp,
            tc.tile_pool(name="dpool", bufs=2) as dpool,
            tc.tile_pool(name="prodp", bufs=2) as prodp,
            tc.tile_pool(name="dyp", bufs=4) as dyp,
            tc.tile_pool(name="pyp", bufs=4) as pyp,
            tc.tile_pool(name="psum_dy", bufs=2, space="PSUM") as psum_dy,
            tc.tile_pool(name="psum_acc", bufs=4, space="PSUM") as psum_acc,
            tc.tile_pool(name="spool", bufs=3) as spool,
        ):
            wt = wp.tile([Y, 4 * Y], F16, name="wt")
            nc.sync.dma_start(out=wt[:, :], in_=w_h[:, :])
            dyT = wt[:, 0:Y]
            ndyT = wt[:, Y : 2 * Y]
            eyeT = wt[:, 2 * Y : 3 * Y]
            eye2T = wt[:, 3 * Y : 4 * Y]

            uvt = inp.tile([Y, 2, D, xh, ZP], F16, name="uvt", tag="uvt")
            # x-split input DMA so early items start while the rest streams
            cuts = [0, KB + 2]
            while cuts[-1] < xh:
                cuts.append(min(cuts[-1] + KB, xh))
            for a, b2 in zip(cuts, cuts[1:]):
                nc.sync.dma_start(out=uvt[:, :, :, a:b2, :],
                                  in_=uv_h[:, :, :, a:b2, :])

            zc = slice(2, 2 + Z)       # center z view
            zp1 = slice(3, 3 + Z)      # z+1
            zm1 = slice(1, 1 + Z)      # z-1

            # Prime PE's vector clock against every input DMA with tiny
            # matmuls, so real matmuls never need a second (DMA) wait —
            # TRN2 matmul instructions support a single sync wait.
            scratch = psum_acc.tile([8, 8], F32, name="scratch", tag="acc")
            for a in cuts[:-1]:
                nc.tensor.matmul(scratch[:, 0:1], wt[:, 0:8],
                                 uvt[:, 0, 0, a, 0:1], start=True, stop=True)

            V, U = 0, 1  # side indices in uvt

            def stage_a(item):
                """Channel-fused diffs + x/z products for one KB-plane item."""
                x0 = item * KB
                u0 = 1 + x0
                xsl = slice(u0, u0 + KB)
                xp1 = slice(u0 + 1, u0 + 1 + KB)
                xm1 = slice(u0 - 1, u0 - 1 + KB)

                # diff tiles [Y, 2, D, KB, Z]: side0 = D~x U, side1 = rev D~x V
                dx = dpool.tile([Y, 2, D, KB, Z], F16, name="dx", tag="dx")
                nc.vector.tensor_sub(out=dx[:, 0, :, :, :],
                                     in0=uvt[:, U, :, xp1, zc],
                                     in1=uvt[:, U, :, xm1, zc])
                nc.vector.tensor_sub(out=dx[:, 1, :, :, :],
                                     in0=uvt[:, V, :, xm1, zc],
                                     in1=uvt[:, V, :, xp1, zc])
                dz = dpool.tile([Y, 2, D, KB, Z], F16, name="dz", tag="dz")
                nc.gpsimd.tensor_sub(out=dz[:, 0, :, :, :],
                                     in0=uvt[:, U, :, xsl, zp1],
                                     in1=uvt[:, U, :, xsl, zm1])
                nc.gpsimd.tensor_sub(out=dz[:, 1, :, :, :],
                                     in0=uvt[:, V, :, xsl, zm1],
                                     in1=uvt[:, V, :, xsl, zp1])

                # products: P[s, i] = dx[s, i] * field(1-s? no: sign-folded so
                # side0 (dU) pairs with V_j and side1 (revdV) with U_j; with
                # uvt side order (V, U): in1 side s uses uvt side s. One 5D op.
                px = prodp.tile([Y, 2, D, KB, Z], F16, name="px", tag="px")
                nc.vector.tensor_mul(
                    out=px[:, :, :, :, :], in0=dx[:, :, :, :, :],
                    in1=uvt[:, :, 0:1, xsl, zc].broadcast_to([Y, 2, D, KB, Z]))
                pz = prodp.tile([Y, 2, D, KB, Z], F16, name="pz", tag="pz")
                # split z-product by side to balance DVE/Pool
                nc.gpsimd.tensor_mul(
                    out=pz[:, 0, :, :, :], in0=dz[:, 0, :, :, :],
                    in1=uvt[:, V, 2:3, xsl, zc].broadcast_to([Y, D, KB, Z]))
                nc.vector.tensor_mul(
                    out=pz[:, 1, :, :, :], in0=dz[:, 1, :, :, :],
                    in1=uvt[:, U, 2:3, xsl, zc].broadcast_to([Y, D, KB, Z]))
                return px, pz

            def stage_b(item, px, pz):
                """Y-diffs/products + accumulation + combine + DMA out."""
                x0 = item * KB
                for h in range(n_h):
                    u0 = 1 + x0 + h * KX
                    hs = slice(u0, u0 + KX)
                    stage = spool.tile([Y, D, KX * Z], F16, name="stage",
                                       tag="stage")
                    for i in range(D):
                        # combine style: Pool-STT for most, PE-inject + Act
                        # evac for a tunable fraction (engine balance)
                        inject = (i == 2)
                        ylr = psum_dy.tile([Y, 2, KX * Z], F32, name="ylr",
                                           tag="ylr")
                        nc.tensor.matmul(ylr[:, 0, :], dyT,
                                         uvt[:, U, i, hs, zc],
                                         start=True, stop=True)
                        nc.tensor.matmul(ylr[:, 1, :], ndyT,
                                         uvt[:, V, i, hs, zc],
                                         start=True, stop=True)
                        # Act evac -> fp16, then DVE product (2x) — cheaper
                        # on the DVE/Pool pair than Pool reading PSUM.
                        dysb = dyp.tile([Y, 2, KX, Z], F16, name="dysb",
                                        tag="dysb")
                        nc.scalar.copy(
                            out=dysb[:, :, :, :].rearrange("p a b c -> p (a b c)"),
                            in_=ylr[:, :, :].rearrange("p a b -> p (a b)"))
                        py = pyp.tile([Y, 2, KX, Z], F16, name="py", tag="py")
                        nc.vector.tensor_mul(out=py[:, :, :, :],
                                             in0=dysb[:, :, :, :],
                                             in1=uvt[:, :, 1, hs, zc])

                        acc = psum_acc.tile([Y, KX * Z], F32, name="acc",
                                            tag="acc")
                        hl = slice(h * KX, h * KX + KX)
                        rhss = [
                            px[:, 0, i, hl, :], px[:, 1, i, hl, :],
                            pz[:, 0, i, hl, :], pz[:, 1, i, hl, :],
                            py[:, 0, :, :], py[:, 1, :, :],
                        ]
                        if inject:
                            rhss.append(uvt[:, U, i, hs, zc])
                        for k, rhs in enumerate(rhss):
                            nc.tensor.matmul(
                                acc[:, :],
                                eye2T if (inject and k == len(rhss) - 1) else eyeT,
                                rhs, start=(k == 0), stop=(k == len(rhss) - 1))
                        if inject:
                            nc.scalar.copy(out=stage[:, i, :], in_=acc[:, :])
                        else:
                            nc.gpsimd.scalar_tensor_tensor(
                                out=stage[:, i, :].rearrange(
                                    "p (a b) -> p a b", a=KX),
                                in0=uvt[:, U, i, hs, zc],
                                scalar=2.0,
                                in1=acc[:, :].rearrange(
                                    "p (a b) -> p a b", a=KX),
                                op0=mybir.AluOpType.mult,
                                op1=mybir.AluOpType.add)
                    nc.sync.dma_start(
                        out=out_h[:, :, x0 + h * KX : x0 + h * KX + KX, :],
                        in_=stage[:, :, :])

            # software pipeline: A(0), A(1), B(0), A(2), B(1), ... B(last)
            prev = None
            prev_p = None
            for item in range(n_items):
                p = stage_a(item)
                if prev is not None:
                    stage_b(prev, *prev_p)
                prev, prev_p = item, p
            stage_b(prev, *prev_p)

    if not nc.is_finalized():
        nc.finalize()
    return nc


def _host_shard(u_b: np.ndarray, v_b: np.ndarray, xs: int) -> list[np.ndarray]:
    """(D, X, Y, Z) f16 pair -> list over x-slabs of [Y, 2, D, xs+2, ZP] f16."""
    shards = []
    for s in range(X // xs):
        idx = (np.arange(-1, xs + 1) + s * xs) % X
        uv = np.empty((Y, 2, D, xs + 2, ZP), dtype=np.float16)
        for side, arr in ((0, v_b), (1, u_b)):
            sl = arr[:, idx, :, :]                  # (D, xs+2, Y, Z)
            sl = np.transpose(sl, (2, 0, 1, 3))     # (Y, D, xs+2, Z)
            uv[:, side, :, :, 2 : 2 + Z] = sl
            uv[:, side, :, :, 0:2] = sl[..., Z - 2 : Z]
            uv[:, side, :, :, 2 + Z :] = sl[..., 0:2]
        shards.append(np.ascontiguousarray(uv))
    return shards


def kernel(left: np.ndarray, right: np.ndarray) -> np.ndarray:
    left = np.asarray(left, dtype=np.float32)
    right = np.asarray(right, dtype=np.float32)
    assert left.shape == (B, D, X, Y, Z), left.shape

    u = ((left + right) * 0.5).astype(np.float16)
    v = ((right - left) * 0.25).astype(np.float16)

    wmats = _make_wmats()
    slabs_per_batch = X // XS  # 4

    shards = [_host_shard(u[b], v[b], XS) for b in range(B)]

    maps = []
    for core in range(NCORES):
        b, s = divmod(core, slabs_per_batch)
        maps.append({"uv": shards[b][s], "wmats": wmats})

    nc = build_nc(XS)
    res = run_bass_kernel_spmd(nc, maps, core_ids=list(range(NCORES)))

    out = np.empty((B, D, X, Y, Z), dtype=np.float32)
    for core in range(NCORES):
        b, s = divmod(core, slabs_per_batch)
        o = res.results[core]["out"]              # (Y, D, XS, Z) f16
        out[b, :, s * XS : (s + 1) * XS, :, :] = np.transpose(
            o.astype(np.float32), (1, 2, 0, 3))
    return out


# ---------------------------------------------------------------------------
# numpy reference of the same math (for probing without jax)
def _np_ref(left: np.ndarray, right: np.ndarray) -> np.ndarray:
    l = np.moveaxis(left, 1, -1).astype(np.float64)
    r = np.moveaxis(right, 1, -1).astype(np.float64)

    def jac(v):
        cols = []
        for j in range(3):
            ax = 1 + j
            g = (np.roll(v, -1, axis=ax) - np.roll(v, 1, axis=ax)) * 0.5
            cols.append(g)
        return np.stack(cols, axis=-1)

    jx, jy = jac(l), jac(r)
    br = np.einsum("bxyzij,bxyzj->bxyzi", jx, r) - np.einsum(
        "bxyzij,bxyzj->bxyzi", jy, l)
    z = l + r + 0.5 * br
    return np.moveaxis(z, -1, 1).astype(np.float32)


if __name__ == "__main__":
    import os
    probe_xs = int(os.environ.get("PROBE_XS", "8"))
    rng = np.random.default_rng(0)
    lf = rng.standard_normal((1, D, X, Y, Z), dtype=np.float32)
    rf = rng.standard_normal((1, D, X, Y, Z), dtype=np.float32)

    u = ((lf[0] + rf[0]) * 0.5).astype(np.float16)
    v = ((rf[0] - lf[0]) * 0.25).astype(np.float16)
    shards = _host_shard(u, v, probe_xs)
    wm = _make_wmats()

    import time
    t0 = time.time()
    nc = build_nc(probe_xs)
    t1 = time.time()
    print(f"build: {t1-t0:.1f}s", flush=True)

    from concourse.bass_interp import CoreSim
    sim = CoreSim(nc)
    sim.tensor("uv")[:] = shards[0]
    sim.tensor("wmats")[:] = wm
    sim.simulate()
    t2 = time.time()
    print(f"sim: {t2-t1:.1f}s  time={int(sim._sim_state.time)}ns", flush=True)

    ref = _np_ref(lf, rf)
    o = np.array(sim.tensor("out"))               # (Y, D, xs, Z) f16
    o = np.transpose(o.astype(np.float32), (1, 2, 0, 3))
    expect = ref[0, :, 0:probe_xs]
    err = np.abs(o - expect)
    rel = np.linalg.norm(o - expect) / np.linalg.norm(expect)
    print(f"rel={rel:.3e} absmax={err.max():.3e} "
          f"out_absmax={np.abs(expect).max():.3f}")


# revision 5
# speedup vs baseline: 1.1515x; 1.1515x over previous
"""Trainium2 Bass kernel for truncated BCH on 3D vector fields (v4).

Math (matches the jax reference):
  out_i = l_i + r_i + 0.25 * sum_j ( D_j l_i * r_j  -  D_j r_i * l_j )
with D_j v = v[.+1] - v[.-1] (circulant wrap) along spatial axis j.

Host re-parameterization (the key algebraic fold):
  U = (l + r)/2,  V = (r - l)/4      (both fp16)
  out_i = 2*U_i + sum_j ( D_j U_i * V_j  -  D_j V_i * U_j )
The 0.25 is absorbed into V (each bracket term is linear in V), the final
scale disappears, and the l+r linear term becomes 2*U, injected either by
the Pool combine (scalar_tensor_tensor) or a 2I matmul.

Sharding: 8 cores = 2 batches x 4 X-slabs of 32 planes (+1 halo plane each
side, wrapped).  Host lays data per core as one uv tensor
[Y, 2(side: 0=V 1=U), D, xh=34, ZP=132] fp16.  Output fp16 [Y, D, 32, Z]
(upcast on host).

Pipeline: flat stream of steps (item, chunk, channel).  PE runs the Y-diff
matmul pair one step ahead of the identity-accumulation matmuls so it
never waits on the Act-evac -> DVE-product chain.  stage_a bulk ops
(channel-fused x/z diffs + products) for item m+1 are interleaved into
item m's step stream.
"""

import sys

sys.path.insert(0, "/opt/trn_rl_repo")

import numpy as np

import concourse.bass as bass
import concourse.bacc as bacc
import concourse.mybir as mybir
import concourse.tile as tile
from concourse.bass_utils import run_bass_kernel_spmd

B, D, X, Y, Z = 2, 3, 128, 128, 128
NCORES = 8
XS = (B * X) // NCORES  # 32 output x-planes per core
ZP = Z + 4              # z padded: [z126, z127, z0..z127, z0, z1]
KX = 4                  # x-planes per psum chunk (acc bank = 512 f32)
ITEMS = (4, 8, 8, 8, 4)  # x-planes per item (sum = XS)

F16 = mybir.dt.float16
F32 = mybir.dt.float32

# --- tuning knobs ----------------------------------------------------------
# per global (chunk, channel) step index k (24 steps):
#   YPATH_A[k]: True -> Act evac + DVE product; False -> Pool reads PSUM
#   COMB_POOL[k]: True -> Pool STT combine; False -> PE 2I inject + Act evac
N_STEPS = (XS // KX) * D
YPATH_A = [True] * N_STEPS
COMB_POOL = [True] * N_STEPS
for _k in range(N_STEPS):
    if _k % 4 == 3:
        YPATH_A[_k] = False          # every 4th step: Pool-direct y-product
    if _k % 2 == 1:
        COMB_POOL[_k] = False        # half the combines: inject + Act evac
# stage_a z-product side assignment: side0 on Pool, side1 on DVE
# ---------------------------------------------------------------------------


def _make_wmats() -> np.ndarray:
    """[dyT | -dyT | I | 2I] as one (Y, 4Y) fp16 matrix (lhsT layout)."""
    e = np.eye(Y, dtype=np.float32)
    dy = np.roll(e, -1, axis=0) - np.roll(e, 1, axis=0)
    dyt = dy.T
    mats = np.concatenate([dyt, -dyt, e, 2.0 * e], axis=1)
    return mats.astype(np.float16)


def build_nc(xs: int = XS) -> bass.Bass:
    xh = xs + 2
    items = list(ITEMS) if xs == XS else [min(8, xs)] * (xs // min(8, xs))
    assert sum(items) == xs
    item_x0 = [sum(items[:m]) for m in range(len(items))]
    nc = bacc.Bacc(None)

    uv_h = nc.declare_dram_parameter("uv", [Y, 2, D, xh, ZP], F16, isOutput=False)
    w_h = nc.declare_dram_parameter("wmats", [Y, 4 * Y], F16, isOutput=False)
    out_h = nc.declare_dram_parameter("out", [Y, D, xs, Z], F16, isOutput=True)

    zc = slice(2, 2 + Z)
    zp1 = slice(3, 3 + Z)
    zm1 = slice(1, 1 + Z)
    V, U = 0, 1  # side indices in uvt

    with tile.TileContext(nc) as tc:
        with (
            tc.tile_pool(name="inp", bufs=1) as inp,
            tc.tile_pool(name="wp", bufs=1) as wp,
            tc.tile_pool(name="dpool", bufs=2) as dpool,
            tc.tile_pool(name="prodp", bufs=2) as prodp,
            tc.tile_pool(name="dyp", bufs=4) as dyp,
            tc.tile_pool(name="pyp", bufs=4) as pyp,
            tc.tile_pool(name="psum_dy", bufs=2, space="PSUM") as psum_dy,
            tc.tile_pool(name="psum_acc", bufs=4, space="PSUM") as psum_acc,
            tc.tile_pool(name="spool", bufs=4) as spool,
        ):
            wt = wp.tile([Y, 4 * Y], F16, name="wt")
            nc.sync.dma_start(out=wt[:, :], in_=w_h[:, :])
            dyT = wt[:, 0:Y]
            ndyT = wt[:, Y : 2 * Y]
            eyeT = wt[:, 2 * Y : 3 * Y]
            eye2T = wt[:, 3 * Y : 4 * Y]

            uvt = inp.tile([Y, 2, D, xh, ZP], F16, name="uvt", tag="uvt")
            # input split per item so early items start while rest streams
            cuts = [0]
            for m, kb in enumerate(items):
                cuts.append(min(item_x0[m] + kb + 2, xh))
            cuts = sorted(set(cuts))
            for a, b2 in zip(cuts, cuts[1:]):
                nc.sync.dma_start(out=uvt[:, :, :, a:b2, :],
                                  in_=uv_h[:, :, :, a:b2, :])

            # prime PE's vector clock against every input DMA (single-wait
            # matmul workaround)
            scratch = psum_acc.tile([8, 8], F32, name="scratch", tag="acc")
            for a in cuts[:-1]:
                nc.tensor.matmul(scratch[:, 0:1], wt[:, 0:8],
                                 uvt[:, 0, 0, a, 0:1], start=True, stop=True)

            def stage_a_ops(m):
                """Return list of (engine_emit_fn) thunks for item m's bulk
                channel-fused diffs + x/z products. Order: DVE ops first."""
                kb = items[m]
                x0 = item_x0[m]
                u0 = 1 + x0
                xsl = slice(u0, u0 + kb)
                xp1 = slice(u0 + 1, u0 + 1 + kb)
                xm1 = slice(u0 - 1, u0 - 1 + kb)
                dx = dpool.tile([Y, 2, D, kb, Z], F16, name="dx", tag="dx")
                dz = dpool.tile([Y, 2, D, kb, Z], F16, name="dz", tag="dz")
                px = prodp.tile([Y, 2, D, kb, Z], F16, name="px", tag="px")
                pz = prodp.tile([Y, 2, D, kb, Z], F16, name="pz", tag="pz")

                def op_dxu():
                    nc.vector.tensor_sub(out=dx[:, 0, :, :, :],
                                         in0=uvt[:, U, :, xp1, zc],
                                         in1=uvt[:, U, :, xm1, zc])

                def op_dxv():
                    nc.vector.tensor_sub(out=dx[:, 1, :, :, :],
                                         in0=uvt[:, V, :, xm1, zc],
                                         in1=uvt[:, V, :, xp1, zc])

                def op_dzu():
                    nc.gpsimd.tensor_sub(out=dz[:, 0, :, :, :],
                                         in0=uvt[:, U, :, xsl, zp1],
                                         in1=uvt[:, U, :, xsl, zm1])

                def op_dzv():
                    nc.gpsimd.tensor_sub(out=dz[:, 1, :, :, :],
                                         in0=uvt[:, V, :, xsl, zm1],
                                         in1=uvt[:, V, :, xsl, zp1])

                def op_px():
                    nc.vector.tensor_mul(
                        out=px[:, :, :, :, :], in0=dx[:, :, :, :, :],
                        in1=uvt[:, :, 0:1, xsl, zc]
                            .broadcast_to([Y, 2, D, kb, Z]))

                def op_pz0():
                    nc.gpsimd.tensor_mul(
                        out=pz[:, 0, :, :, :], in0=dz[:, 0, :, :, :],
                        in1=uvt[:, V, 2:3, xsl, zc]
                            .broadcast_to([Y, D, kb, Z]))

                def op_pz1():
                    nc.vector.tensor_mul(
                        out=pz[:, 1, :, :, :], in0=dz[:, 1, :, :, :],
                        in1=uvt[:, U, 2:3, xsl, zc]
                            .broadcast_to([Y, D, kb, Z]))

                # emission order within the interleave: DVE deps first
                ops = [op_dxu, op_dxv, op_dzu, op_dzv, op_px, op_pz0, op_pz1]
                return (px, pz), ops

            # flat step list: (item, h, i) with global chunk x-offset
            steps = []
            for m, kb in enumerate(items):
                for h in range(kb // KX):
                    for i in range(D):
                        steps.append((m, item_x0[m] + h * KX, h, i))

            def emit_dy(k):
                m, xg, h, i = steps[k]
                u0 = 1 + xg
                hs = slice(u0, u0 + KX)
                ylr = psum_dy.tile([Y, 2, KX * Z], F32, name="ylr", tag="ylr")
                nc.tensor.matmul(ylr[:, 0, :], dyT, uvt[:, U, i, hs, zc],
                                 start=True, stop=True)
                nc.tensor.matmul(ylr[:, 1, :], ndyT, uvt[:, V, i, hs, zc],
                                 start=True, stop=True)
                return ylr

            prods = {}   # item -> (px, pz)
            pend_a = []  # pending stage_a op thunks
            stage_tiles = {}

            def emit_step(k, ylr_k):
                m, xg, h, i = steps[k]
                u0 = 1 + xg
                hs = slice(u0, u0 + KX)
                px, pz = prods[m]
                hl = slice(h * KX, h * KX + KX)
                # y-path
                if YPATH_A[k]:
                    dysb = dyp.tile([Y, 2, KX, Z], F16, name="dysb",
                                    tag="dysb")
                    nc.scalar.copy(
                        out=dysb[:, :, :, :].rearrange("p a b c -> p (a b c)"),
                        in_=ylr_k[:, :, :].rearrange("p a b -> p (a b)"))
                    py = pyp.tile([Y, 2, KX, Z], F16, name="py", tag="py")
                    nc.vector.tensor_mul(out=py[:, :, :, :],
                                         in0=dysb[:, :, :, :],
                                         in1=uvt[:, :, 1, hs, zc])
                else:
                    py = pyp.tile([Y, 2, KX, Z], F16, name="py", tag="py")
                    nc.gpsimd.tensor_mul(
                        out=py[:, :, :, :],
                        in0=ylr_k[:, :, :].rearrange(
                            "p a (b c) -> p a b c", b=KX),
                        in1=uvt[:, :, 1, hs, zc])
                # one pending stage_a bulk op rides along mid-step
                if pend_a:
                    pend_a.pop(0)()
                # accumulate
                acc = psum_acc.tile([Y, KX * Z], F32, name="acc", tag="acc")
                rhss = [px[:, 0, i, hl, :], px[:, 1, i, hl, :],
                        pz[:, 0, i, hl, :], pz[:, 1, i, hl, :],
                        py[:, 0, :, :], py[:, 1, :, :]]
                if not COMB_POOL[k]:
                    rhss.append(uvt[:, U, i, hs, zc])
                for kk, rhs in enumerate(rhss):
                    w = eye2T if (not COMB_POOL[k] and kk == len(rhss) - 1) \
                        else eyeT
                    nc.tensor.matmul(acc[:, :], w, rhs, start=(kk == 0),
                                     stop=(kk == len(rhss) - 1))
                # combine -> stage
                key = (m, h)
                if key not in stage_tiles:
                    stage_tiles[key] = spool.tile([Y, D, KX * Z], F16,
                                                  name="stage", tag="stage")
                stage = stage_tiles[key]
                if COMB_POOL[k]:
                    nc.gpsimd.scalar_tensor_tensor(
                        out=stage[:, i, :].rearrange("p (a b) -> p a b", a=KX),
                        in0=uvt[:, U, i, hs, zc], scalar=2.0,
                        in1=acc[:, :].rearrange("p (a b) -> p a b", a=KX),
                        op0=mybir.AluOpType.mult, op1=mybir.AluOpType.add)
                else:
                    nc.scalar.copy(out=stage[:, i, :], in_=acc[:, :])
                if pend_a:
                    pend_a.pop(0)()
                if i == D - 1:
                    nc.sync.dma_start(out=out_h[:, :, xg : xg + KX, :],
                                      in_=stage[:, :, :])
                    del stage_tiles[key]

            # prologue: stage_a(item0) fully, then the step stream with
            # item m+1's stage_a interleaved into item m's steps.
            p0, ops0 = stage_a_ops(0)
            prods[0] = p0
            for op in ops0:
                op()
            next_item = 1
            ylr_next = emit_dy(0)
            for k in range(len(steps)):
                m = steps[k][0]
                # queue next item's stage_a once we enter item m's steps
                if next_item <= m + 1 and next_item < len(items):
                    p, ops = stage_a_ops(next_item)
                    prods[next_item] = p
                    pend_a.extend(ops)
                    next_item += 1
                ylr_k = ylr_next
                if k + 1 < len(steps):
                    ylr_next = emit_dy(k + 1)
                emit_step(k, ylr_k)
            # flush any leftover stage_a thunks (shouldn't happen)
            for op in pend_a:
                op()

    if not nc.is_finalized():
        nc.finalize()
    return nc


def _host_shard(u_b: np.ndarray, v_b: np.ndarray, xs: int) -> list[np.ndarray]:
    """(D, X, Y, Z) f16 pair -> list over x-slabs of [Y, 2, D, xs+2, ZP] f16."""
    shards = []
    for s in range(X // xs):
        idx = (np.arange(-1, xs + 1) + s * xs) % X
        uv = np.empty((Y, 2, D, xs + 2, ZP), dtype=np.float16)
        for side, arr in ((0, v_b), (1, u_b)):
            sl = arr[:, idx, :, :]                  # (D, xs+2, Y, Z)
            sl = np.transpose(sl, (2, 0, 1, 3))     # (Y, D, xs+2, Z)
            uv[:, side, :, :, 2 : 2 + Z] = sl
            uv[:, side, :, :, 0:2] = sl[..., Z - 2 : Z]
            uv[:, side, :, :, 2 + Z :] = sl[..., 0:2]
        shards.append(np.ascontiguousarray(uv))
    return shards


def kernel(left: np.ndarray, right: np.ndarray) -> np.ndarray:
    left = np.asarray(left, dtype=np.float32)
    right = np.asarray(right, dtype=np.float32)
    assert left.shape == (B, D, X, Y, Z), left.shape

    u = ((left + right) * 0.5).astype(np.float16)
    v = ((right - left) * 0.25).astype(np.float16)

    wmats = _make_wmats()
    slabs_per_batch = X // XS  # 4

    shards = [_host_shard(u[b], v[b], XS) for b in range(B)]

    maps = []
    for core in range(NCORES):
        b, s = divmod(core, slabs_per_batch)
        maps.append({"uv": shards[b][s], "wmats": wmats})

    nc = build_nc(XS)
    res = run_bass_kernel_spmd(nc, maps, core_ids=list(range(NCORES)))

    out = np.empty((B, D, X, Y, Z), dtype=np.float32)
    for core in range(NCORES):
        b, s = divmod(core, slabs_per_batch)
        o = res.results[core]["out"]              # (Y, D, XS, Z) f16
        out[b, :, s * XS : (s + 1) * XS, :, :] = np.transpose(
            o.astype(np.float32), (1, 2, 0, 3))
    return out


# ---------------------------------------------------------------------------
def _np_ref(left: np.ndarray, right: np.ndarray) -> np.ndarray:
    l = np.moveaxis(left, 1, -1).astype(np.float64)
    r = np.moveaxis(right, 1, -1).astype(np.float64)

    def jac(v):
        cols = []
        for j in range(3):
            ax = 1 + j
            g = (np.roll(v, -1, axis=ax) - np.roll(v, 1, axis=ax)) * 0.5
            cols.append(g)
        return np.stack(cols, axis=-1)

    jx, jy = jac(l), jac(r)
    br = np.einsum("bxyzij,bxyzj->bxyzi", jx, r) - np.einsum(
        "bxyzij,bxyzj->bxyzi", jy, l)
    z = l + r + 0.5 * br
    return np.moveaxis(z, -1, 1).astype(np.float32)


if __name__ == "__main__":
    import os
    probe_xs = int(os.environ.get("PROBE_XS", "8"))
    rng = np.random.default_rng(0)
    lf = rng.standard_normal((1, D, X, Y, Z), dtype=np.float32)
    rf = rng.standard_normal((1, D, X, Y, Z), dtype=np.float32)

    u = ((lf[0] + rf[0]) * 0.5).astype(np.float16)
    v = ((rf[0] - lf[0]) * 0.25).astype(np.float16)
    shards = _host_shard(u, v, probe_xs)
    wm = _make_wmats()

    import time
    t0 = time.time()
    nc = build_nc(probe_xs)
    t1 = time.time()
    print(f"build: {t1-t0:.1f}s", flush=True)

    from concourse.bass_interp import CoreSim
    sim = CoreSim(nc)
    sim.tensor("uv")[:] = shards[0]
    sim.tensor("wmats")[:] = wm
    sim.simulate()
    t2 = time.time()
    print(f"sim: {t2-t1:.1f}s  time={int(sim._sim_state.time)}ns", flush=True)

    ref = _np_ref(lf, rf)
    o = np.array(sim.tensor("out"))               # (Y, D, xs, Z) f16
    o = np.transpose(o.astype(np.float32), (1, 2, 0, 3))
    expect = ref[0, :, 0:probe_xs]
    err = np.abs(o - expect)
    rel = np.linalg.norm(o - expect) / np.linalg.norm(expect)
    print(f"rel={rel:.3e} absmax={err.max():.3e} "
          f"out_absmax={np.abs(expect).max():.3f}")
